# revision 12
# baseline (speedup 1.0000x reference)
"""Trainium2 Bass kernel for nn_RandomDropout (B=8192, S=2048, int32 ids).

Semantics (training mode of the module):
  - rows with odd batch index and n_tokens >= 10 get ONE random valid
    position dropped (argmin of jax.random.uniform(key 42) over the valid
    prefix), then the row is compacted left (shift-by-one from the drop
    position, zero appended at the end of the valid region).
  - all other rows pass through unchanged.

Device strategy (pure data parallel over 8 NeuronCores):
  - Only odd rows can ever change -> only the 4096 odd rows are processed
    on device (512 rows per core); even rows are copied host-side.
  - Token ids are < 32000, so rows ship to the device as int16 (halves
    HBM traffic).
  - The random matrix r = uniform(key42, (8192, 2048)) is input-independent,
    so its per-row prefix-minimum "record" positions are precomputed on the
    host (max 18 records/row, padded to K=32) and shipped as a tiny constant.
  - Per 128-row tile, work is spread over three engines:
      ACT:    n_tokens = accumulate(sign(x))
      DVE:    thresh = max over k of rec_k*(rec_k < nt), plus the
              (n_tokens < 10 -> no drop) override; in-place predicated
              left-shift-by-one (the compaction)
      GpSimd: mask = (iota >= thresh), iota generated on-chip
"""

import os
import sys

import numpy as np

_TRN_REPO = "/opt/trn_rl_repo"
if os.path.isdir(_TRN_REPO) and _TRN_REPO not in sys.path:  # pragma: no cover
    sys.path.insert(0, _TRN_REPO)

B, S = 8192, 2048
N_CORES = 8
MIN_TOKENS = 10
K = 32  # padded record-position count (measured max is 18)
ROWS_PER_CORE = (B // 2) // N_CORES  # 512 odd rows per core
TILES_PER_CORE = ROWS_PER_CORE // 128  # 4 tiles of [128, 2048]

_cache = {}


def _records_odd() -> np.ndarray:
    """[4096, K] int32: prefix-min record positions of r's odd rows, pad=S."""
    if "rec" in _cache:
        return _cache["rec"]
    import jax

    with jax.default_device(jax.devices("cpu")[0]):
        key = jax.random.key(42)
        r = np.asarray(jax.random.uniform(key, (B, S)))
    r_odd = r[1::2]
    pm = np.minimum.accumulate(r_odd, axis=1)
    is_rec = np.empty(r_odd.shape, dtype=bool)
    is_rec[:, 0] = True
    is_rec[:, 1:] = r_odd[:, 1:] < pm[:, :-1]
    assert int(is_rec.sum(1).max()) <= K
    rec = np.full((B // 2, K), S, np.int32)
    cc = np.cumsum(is_rec, axis=1) - 1
    rows, cols = np.nonzero(is_rec)
    rec[rows, cc[rows, cols]] = cols
    _cache["rec"] = rec
    return rec


def _build_program():
    """Build the single-core Bass program (SPMD across 8 cores)."""
    import concourse.bacc as bacc
    import concourse.mybir as mybir
    import concourse.tile as tile

    i16 = mybir.dt.int16
    i32 = mybir.dt.int32
    f32 = mybir.dt.float32
    Alu = mybir.AluOpType

    # Bacc (not plain Bass): its compile() splits multi-sem waits into the
    # event-semaphore form the TRN2 sequencers require (<=1 wait per inst).
    nc = bacc.Bacc(
        "TRN2",
        target_bir_lowering=False,
        debug=False,
        enable_asserts=False,
        num_devices=N_CORES,
    )
    x_d = nc.dram_tensor("x", [ROWS_PER_CORE, S], i16, kind="ExternalInput").ap()
    # records, interleaved so tile t / partition p reads row t*128+p:
    # host layout [128, TILES_PER_CORE * K]
    rec_d = nc.dram_tensor(
        "rec", [128, TILES_PER_CORE * K], i32, kind="ExternalInput"
    ).ap()
    y_d = nc.dram_tensor("y", [ROWS_PER_CORE, S], i16, kind="ExternalOutput").ap()

    with tile.TileContext(nc) as tc:
        with (
            tc.tile_pool(name="const", bufs=1) as const_pool,
            tc.tile_pool(name="xin", bufs=4) as xin_pool,
            tc.tile_pool(name="mask", bufs=4) as mask_pool,
            tc.tile_pool(name="scr", bufs=4) as scr_pool,
            tc.tile_pool(name="small", bufs=8) as small_pool,
        ):
            # x tile 0 first: it gates the whole pipeline
            xts = [
                xin_pool.tile([128, S + 1], i16, tag="xt", name=f"xt{t}")
                for t in range(TILES_PER_CORE)
            ]
            for t in range(TILES_PER_CORE):
                nc.sync.dma_start(
                    xts[t][:, 0:S], x_d[t * 128 : (t + 1) * 128, :]
                )
            rec_sb = const_pool.tile([128, TILES_PER_CORE * K], i32, tag="rec")
            nc.sync.dma_start(rec_sb[:], rec_d[:])
            iota = const_pool.tile([128, S], i16, tag="iota")
            nc.gpsimd.iota(iota[:], pattern=[[1, S]], base=0, channel_multiplier=0)
            for t in range(TILES_PER_CORE):
                nc.vector.memset(xts[t][:, S : S + 1], 0)

            for t in range(TILES_PER_CORE):
                xt = xts[t]

                # n_tokens = sum(sign(x)) on the scalar engine (ids >= 0)
                scr = scr_pool.tile([128, S], i16)
                nt = small_pool.tile([128, 1], f32, tag="nt")
                nc.scalar.activation(
                    scr[:],
                    xt[:, 0:S],
                    mybir.ActivationFunctionType.Sign,
                    accum_out=nt[:],
                )

                # thresh = max( max_k rec_k*(rec_k < nt), (x[:,9]==0)*S )
                rt = rec_sb[:, t * K : (t + 1) * K]
                tmp = small_pool.tile([128, K + 1], i32, tag="tmp")
                nc.vector.scalar_tensor_tensor(
                    tmp[:, 0:K], rt, nt[:], rt, Alu.is_lt, Alu.mult
                )
                nc.vector.tensor_scalar(
                    tmp[:, K : K + 1],
                    xt[:, MIN_TOKENS - 1 : MIN_TOKENS],
                    0.0,
                    float(S),
                    Alu.is_equal,
                    Alu.mult,
                )
                thr = small_pool.tile([128, 1], f32, tag="thr")
                nc.vector.tensor_reduce(
                    thr[:], tmp[:], mybir.AxisListType.X, Alu.max
                )

                # mask = (iota >= thresh) on gpsimd; predicated in-place shift
                mk = mask_pool.tile([128, S], i16)
                nc.gpsimd.tensor_scalar(
                    mk[:], iota[:], thr[:], None, Alu.is_ge, Alu.bypass
                )
                nc.vector.copy_predicated(xt[:, 0:S], mk[:], xt[:, 1 : S + 1])
                nc.sync.dma_start(y_d[t * 128 : (t + 1) * 128, :], xt[:, 0:S])

    nc.finalize()
    return nc


def _get_program():
    if "nc" not in _cache:
        _cache["nc"] = _build_program()
    return _cache["nc"]


def _shard_inputs(input_ids: np.ndarray):
    """Per-core in_maps: odd rows of the core's contiguous 1024-row block."""
    rec = _records_odd()
    odd16 = input_ids[1::2].astype(np.int16)  # ids < 32000 fit int16
    in_maps = []
    for c in range(N_CORES):
        sl = slice(c * ROWS_PER_CORE, (c + 1) * ROWS_PER_CORE)
        rec_c = (
            rec[sl]
            .reshape(TILES_PER_CORE, 128, K)
            .transpose(1, 0, 2)
            .reshape(128, TILES_PER_CORE * K)
        )
        in_maps.append(
            {
                "x": np.ascontiguousarray(odd16[sl]),
                "rec": np.ascontiguousarray(rec_c),
            }
        )
    return in_maps


def _run(input_ids: np.ndarray, trace: bool = False):
    from concourse.bass_utils import run_bass_kernel_spmd

    nc = _get_program()
    in_maps = _shard_inputs(input_ids)
    return run_bass_kernel_spmd(nc, in_maps, list(range(N_CORES)), trace=trace)


def kernel(input_ids: np.ndarray) -> np.ndarray:
    input_ids = np.ascontiguousarray(np.asarray(input_ids, dtype=np.int32))
    assert input_ids.shape == (B, S)
    res = _run(input_ids)
    out = input_ids.copy()
    out[1::2] = np.concatenate([m["y"] for m in res.results], axis=0)
    return out


# revision 13
# speedup vs baseline: 4.2097x; 4.2097x over previous
"""Trainium2 Bass kernel for nn_RandomDropout (B=8192, S=2048, int32 ids).

Semantics (training mode of the module):
  - rows with odd batch index and n_tokens >= 10 get ONE random valid
    position dropped (argmin of jax.random.uniform(key 42) over the valid
    prefix), then the row is compacted left (shift-by-one from the drop
    position, zero appended at the end of the valid region).
  - all other rows pass through unchanged.

Device strategy (pure data parallel over 8 NeuronCores):
  - Only odd rows can ever change -> only the 4096 odd rows are processed
    on device (512 rows per core); even rows are copied host-side.
  - Token ids are < 32000, so rows ship to the device as int16 (halves
    HBM traffic).
  - The random matrix r = uniform(key42, (8192, 2048)) is input-independent,
    so its per-row prefix-minimum "record" positions are precomputed on the
    host (max 18 records/row, padded to K=32) and shipped as a tiny constant.
  - Per 128-row tile, work is spread over three engines:
      ACT:    n_tokens = accumulate(sign(x))
      DVE:    thresh = max over k of rec_k*(rec_k < nt), plus the
              (n_tokens < 10 -> no drop) override; in-place predicated
              left-shift-by-one (the compaction)
      DVE:    mask = (iota >= thresh)  (GpSimd is ~15x too slow for this)
"""

import os
import sys

import numpy as np

_TRN_REPO = "/opt/trn_rl_repo"
if os.path.isdir(_TRN_REPO) and _TRN_REPO not in sys.path:  # pragma: no cover
    sys.path.insert(0, _TRN_REPO)

B, S = 8192, 2048
N_CORES = 8
MIN_TOKENS = 10
K = 32  # padded record-position count (measured max is 18)
ROWS_PER_CORE = (B // 2) // N_CORES  # 512 odd rows per core
TILES_PER_CORE = ROWS_PER_CORE // 128  # 4 tiles of [128, 2048]

_cache = {}


def _records_odd() -> np.ndarray:
    """[4096, K] int32: prefix-min record positions of r's odd rows, pad=S."""
    if "rec" in _cache:
        return _cache["rec"]
    import jax

    with jax.default_device(jax.devices("cpu")[0]):
        key = jax.random.key(42)
        r = np.asarray(jax.random.uniform(key, (B, S)))
    r_odd = r[1::2]
    pm = np.minimum.accumulate(r_odd, axis=1)
    is_rec = np.empty(r_odd.shape, dtype=bool)
    is_rec[:, 0] = True
    is_rec[:, 1:] = r_odd[:, 1:] < pm[:, :-1]
    assert int(is_rec.sum(1).max()) <= K
    rec = np.full((B // 2, K), S, np.int32)
    cc = np.cumsum(is_rec, axis=1) - 1
    rows, cols = np.nonzero(is_rec)
    rec[rows, cc[rows, cols]] = cols
    _cache["rec"] = rec
    return rec


def _build_program():
    """Build the single-core Bass program (SPMD across 8 cores)."""
    import concourse.bacc as bacc
    import concourse.mybir as mybir
    import concourse.tile as tile

    i16 = mybir.dt.int16
    i32 = mybir.dt.int32
    f32 = mybir.dt.float32
    Alu = mybir.AluOpType

    # Bacc (not plain Bass): its compile() splits multi-sem waits into the
    # event-semaphore form the TRN2 sequencers require (<=1 wait per inst).
    nc = bacc.Bacc(
        "TRN2",
        target_bir_lowering=False,
        debug=False,
        enable_asserts=False,
        num_devices=N_CORES,
    )
    x_d = nc.dram_tensor("x", [ROWS_PER_CORE, S], i16, kind="ExternalInput").ap()
    # records, interleaved so tile t / partition p reads row t*128+p:
    # host layout [128, TILES_PER_CORE * K]
    rec_d = nc.dram_tensor(
        "rec", [128, TILES_PER_CORE * K], i32, kind="ExternalInput"
    ).ap()
    iota_d = nc.dram_tensor("iota", [128, S], i16, kind="ExternalInput").ap()
    y_d = nc.dram_tensor("y", [ROWS_PER_CORE, S], i16, kind="ExternalOutput").ap()

    with tile.TileContext(nc) as tc:
        with (
            tc.tile_pool(name="const", bufs=1) as const_pool,
            tc.tile_pool(name="xin", bufs=4) as xin_pool,
            tc.tile_pool(name="mask", bufs=4) as mask_pool,
            tc.tile_pool(name="scr", bufs=4) as scr_pool,
            tc.tile_pool(name="small", bufs=8) as small_pool,
        ):
            # x tile 0 first: it gates the whole pipeline
            xts = [
                xin_pool.tile([128, S + 1], i16, tag="xt", name=f"xt{t}")
                for t in range(TILES_PER_CORE)
            ]
            for t in range(TILES_PER_CORE):
                nc.sync.dma_start(
                    xts[t][:, 0:S], x_d[t * 128 : (t + 1) * 128, :]
                )
            rec_sb = const_pool.tile([128, TILES_PER_CORE * K], i32, tag="rec")
            nc.sync.dma_start(rec_sb[:], rec_d[:])
            iota = const_pool.tile([128, S], i16, tag="iota")
            nc.sync.dma_start(iota[:], iota_d[:])
            for t in range(TILES_PER_CORE):
                nc.vector.memset(xts[t][:, S : S + 1], 0)

            for t in range(TILES_PER_CORE):
                xt = xts[t]

                # n_tokens = sum(sign(x)) on the scalar engine (ids >= 0)
                scr = scr_pool.tile([128, S], i16)
                nt = small_pool.tile([128, 1], f32, tag="nt")
                nc.scalar.activation(
                    scr[:],
                    xt[:, 0:S],
                    mybir.ActivationFunctionType.Sign,
                    accum_out=nt[:],
                )

                # thresh = max( max_k rec_k*(rec_k < nt), (x[:,9]==0)*S )
                rt = rec_sb[:, t * K : (t + 1) * K]
                tmp = small_pool.tile([128, K + 1], i32, tag="tmp")
                nc.vector.scalar_tensor_tensor(
                    tmp[:, 0:K], rt, nt[:], rt, Alu.is_lt, Alu.mult
                )
                nc.vector.tensor_scalar(
                    tmp[:, K : K + 1],
                    xt[:, MIN_TOKENS - 1 : MIN_TOKENS],
                    0.0,
                    float(S),
                    Alu.is_equal,
                    Alu.mult,
                )
                thr = small_pool.tile([128, 1], f32, tag="thr")
                nc.vector.tensor_reduce(
                    thr[:], tmp[:], mybir.AxisListType.X, Alu.max
                )

                # mask = (iota >= thresh) on gpsimd; predicated in-place shift
                mk = mask_pool.tile([128, S], i16)
                nc.vector.tensor_scalar(
                    mk[:], iota[:], thr[:], None, Alu.is_ge, Alu.bypass
                )
                nc.vector.copy_predicated(xt[:, 0:S], mk[:], xt[:, 1 : S + 1])
                nc.sync.dma_start(y_d[t * 128 : (t + 1) * 128, :], xt[:, 0:S])

    nc.finalize()
    return nc


def _get_program():
    if "nc" not in _cache:
        _cache["nc"] = _build_program()
    return _cache["nc"]


def _shard_inputs(input_ids: np.ndarray):
    """Per-core in_maps: odd rows of the core's contiguous 1024-row block."""
    rec = _records_odd()
    odd16 = input_ids[1::2].astype(np.int16)  # ids < 32000 fit int16
    iota = np.broadcast_to(np.arange(S, dtype=np.int16), (128, S)).copy()
    in_maps = []
    for c in range(N_CORES):
        sl = slice(c * ROWS_PER_CORE, (c + 1) * ROWS_PER_CORE)
        rec_c = (
            rec[sl]
            .reshape(TILES_PER_CORE, 128, K)
            .transpose(1, 0, 2)
            .reshape(128, TILES_PER_CORE * K)
        )
        in_maps.append(
            {
                "x": np.ascontiguousarray(odd16[sl]),
                "rec": np.ascontiguousarray(rec_c),
                "iota": iota,
            }
        )
    return in_maps


def _run(input_ids: np.ndarray, trace: bool = False):
    from concourse.bass_utils import run_bass_kernel_spmd

    nc = _get_program()
    in_maps = _shard_inputs(input_ids)
    return run_bass_kernel_spmd(nc, in_maps, list(range(N_CORES)), trace=trace)


def kernel(input_ids: np.ndarray) -> np.ndarray:
    input_ids = np.ascontiguousarray(np.asarray(input_ids, dtype=np.int32))
    assert input_ids.shape == (B, S)
    res = _run(input_ids)
    out = input_ids.copy()
    out[1::2] = np.concatenate([m["y"] for m in res.results], axis=0)
    return out


# revision 17
# speedup vs baseline: 4.2130x; 1.0008x over previous
"""Trainium2 Bass kernel for nn_RandomDropout (B=8192, S=2048, int32 ids).

Semantics (training mode of the module):
  - rows with odd batch index and n_tokens >= 10 get ONE random valid
    position dropped (argmin of jax.random.uniform(key 42) over the valid
    prefix), then the row is compacted left (shift-by-one from the drop
    position, zero appended at the end of the valid region).
  - all other rows pass through unchanged.

Device strategy (pure data parallel over 8 NeuronCores):
  - Only odd rows can ever change -> only the 4096 odd rows are processed
    on device (512 rows per core); even rows are copied host-side.
  - Token ids are < 32000, so rows ship to the device as int16 (halves
    HBM traffic).
  - The random matrix r = uniform(key42, (8192, 2048)) is input-independent,
    so its per-row prefix-minimum "record" positions are precomputed on the
    host (max 18 records/row, padded to K=32) and shipped as a tiny constant.
  - Per 128-row tile, work is spread over three engines:
      ACT:    n_tokens = accumulate(sign(x))
      DVE:    thresh = max over k of rec_k*(rec_k < nt), plus the
              (n_tokens < 10 -> no drop) override; in-place predicated
              left-shift-by-one (the compaction)
      DVE:    mask = (iota >= thresh)  (GpSimd is ~15x too slow for this)
"""

import os
import sys

import numpy as np

_TRN_REPO = "/opt/trn_rl_repo"
if os.path.isdir(_TRN_REPO) and _TRN_REPO not in sys.path:  # pragma: no cover
    sys.path.insert(0, _TRN_REPO)

B, S = 8192, 2048
N_CORES = 8
MIN_TOKENS = 10
K = 32  # padded record-position count (measured max is 18)
ROWS_PER_CORE = (B // 2) // N_CORES  # 512 odd rows per core
TILES_PER_CORE = ROWS_PER_CORE // 128  # 4 tiles of [128, 2048]

_cache = {}


def _records_odd() -> np.ndarray:
    """[4096, K] int32: prefix-min record positions of r's odd rows, pad=S."""
    if "rec" in _cache:
        return _cache["rec"]
    import jax

    with jax.default_device(jax.devices("cpu")[0]):
        key = jax.random.key(42)
        r = np.asarray(jax.random.uniform(key, (B, S)))
    r_odd = r[1::2]
    pm = np.minimum.accumulate(r_odd, axis=1)
    is_rec = np.empty(r_odd.shape, dtype=bool)
    is_rec[:, 0] = True
    is_rec[:, 1:] = r_odd[:, 1:] < pm[:, :-1]
    assert int(is_rec.sum(1).max()) <= K
    rec = np.full((B // 2, K), S, np.int32)
    cc = np.cumsum(is_rec, axis=1) - 1
    rows, cols = np.nonzero(is_rec)
    rec[rows, cc[rows, cols]] = cols
    _cache["rec"] = rec
    return rec


def _build_program():
    """Build the single-core Bass program (SPMD across 8 cores)."""
    import concourse.bacc as bacc
    import concourse.mybir as mybir
    import concourse.tile as tile

    i16 = mybir.dt.int16
    i32 = mybir.dt.int32
    f32 = mybir.dt.float32
    Alu = mybir.AluOpType

    # Bacc (not plain Bass): its compile() splits multi-sem waits into the
    # event-semaphore form the TRN2 sequencers require (<=1 wait per inst).
    nc = bacc.Bacc(
        "TRN2",
        target_bir_lowering=False,
        debug=False,
        enable_asserts=False,
        num_devices=N_CORES,
    )
    x_d = nc.dram_tensor("x", [ROWS_PER_CORE, S], i16, kind="ExternalInput").ap()
    # records, interleaved so tile t / partition p reads row t*128+p:
    # host layout [128, TILES_PER_CORE * K]
    rec_d = nc.dram_tensor(
        "rec", [128, TILES_PER_CORE * K], i32, kind="ExternalInput"
    ).ap()
    iota_d = nc.dram_tensor("iota", [128, S], i16, kind="ExternalInput").ap()
    y_d = nc.dram_tensor("y", [ROWS_PER_CORE, S], i16, kind="ExternalOutput").ap()

    with tile.TileContext(nc) as tc:
        with (
            tc.tile_pool(name="const", bufs=1) as const_pool,
            tc.tile_pool(name="xin", bufs=4) as xin_pool,
            tc.tile_pool(name="mask", bufs=4) as mask_pool,
            tc.tile_pool(name="scr", bufs=4) as scr_pool,
            tc.tile_pool(name="small", bufs=8) as small_pool,
        ):
            # x tile 0 first: it gates the whole pipeline
            xts = [
                xin_pool.tile([128, S + 1], i16, tag="xt", name=f"xt{t}")
                for t in range(TILES_PER_CORE)
            ]
            for t in range(TILES_PER_CORE):
                nc.sync.dma_start(
                    xts[t][:, 0:S], x_d[t * 128 : (t + 1) * 128, :]
                )
            rec_sb = const_pool.tile([128, TILES_PER_CORE * K], i32, tag="rec")
            nc.sync.dma_start(rec_sb[:], rec_d[:])
            iota = const_pool.tile([128, S], i16, tag="iota")
            nc.sync.dma_start(iota[:], iota_d[:])
            for t in range(TILES_PER_CORE):
                nc.vector.memset(xts[t][:, S : S + 1], 0)

            for t in range(TILES_PER_CORE):
                xt = xts[t]

                # n_tokens = sum(sign(x)) on the scalar engine (ids >= 0)
                scr = scr_pool.tile([128, S], i16)
                nt = small_pool.tile([128, 1], f32, tag="nt")
                nc.scalar.activation(
                    scr[:],
                    xt[:, 0:S],
                    mybir.ActivationFunctionType.Sign,
                    accum_out=nt[:],
                )

                # thresh = max( max_k rec_k*(rec_k < nt), (x[:,9]==0)*S )
                rt = rec_sb[:, t * K : (t + 1) * K]
                tmp = small_pool.tile([128, K + 1], i32, tag="tmp")
                nc.vector.scalar_tensor_tensor(
                    tmp[:, 0:K], rt, nt[:], rt, Alu.is_lt, Alu.mult
                )
                nc.vector.tensor_scalar(
                    tmp[:, K : K + 1],
                    xt[:, MIN_TOKENS - 1 : MIN_TOKENS],
                    0.0,
                    float(S),
                    Alu.is_equal,
                    Alu.mult,
                )
                thr = small_pool.tile([128, 1], f32, tag="thr")
                nc.vector.tensor_reduce(
                    thr[:], tmp[:], mybir.AxisListType.X, Alu.max
                )

                # mask = (iota >= thresh) on DVE; predicated in-place shift.
                # (An ACT relu(iota-thr+1) variant for the mask wedged the
                # execution unit on HW — keep the mask on DVE.)
                mk = mask_pool.tile([128, S], i16)
                nc.vector.tensor_scalar(
                    mk[:], iota[:], thr[:], None, Alu.is_ge, Alu.bypass
                )
                nc.vector.copy_predicated(xt[:, 0:S], mk[:], xt[:, 1 : S + 1])
                nc.sync.dma_start(y_d[t * 128 : (t + 1) * 128, :], xt[:, 0:S])

    nc.finalize()
    return nc


def _get_program():
    if "nc" not in _cache:
        _cache["nc"] = _build_program()
    return _cache["nc"]


def _shard_inputs(input_ids: np.ndarray):
    """Per-core in_maps: odd rows of the core's contiguous 1024-row block."""
    rec = _records_odd()
    odd16 = input_ids[1::2].astype(np.int16)  # ids < 32000 fit int16
    iota = np.broadcast_to(np.arange(S, dtype=np.int16), (128, S)).copy()
    in_maps = []
    for c in range(N_CORES):
        sl = slice(c * ROWS_PER_CORE, (c + 1) * ROWS_PER_CORE)
        rec_c = (
            rec[sl]
            .reshape(TILES_PER_CORE, 128, K)
            .transpose(1, 0, 2)
            .reshape(128, TILES_PER_CORE * K)
        )
        in_maps.append(
            {
                "x": np.ascontiguousarray(odd16[sl]),
                "rec": np.ascontiguousarray(rec_c),
                "iota": iota,
            }
        )
    return in_maps


def _run(input_ids: np.ndarray, trace: bool = False):
    from concourse.bass_utils import run_bass_kernel_spmd

    nc = _get_program()
    in_maps = _shard_inputs(input_ids)
    return run_bass_kernel_spmd(nc, in_maps, list(range(N_CORES)), trace=trace)


def kernel(input_ids: np.ndarray) -> np.ndarray:
    input_ids = np.ascontiguousarray(np.asarray(input_ids, dtype=np.int32))
    assert input_ids.shape == (B, S)
    res = _run(input_ids)
    out = input_ids.copy()
    out[1::2] = np.concatenate([m["y"] for m in res.results], axis=0)
    return out


# revision 18
# speedup vs baseline: 4.5924x; 1.0901x over previous
"""Trainium2 Bass kernel for nn_RandomDropout (B=8192, S=2048, int32 ids).

Semantics (training mode of the module):
  - rows with odd batch index and n_tokens >= 10 get ONE random valid
    position dropped (argmin of jax.random.uniform(key 42) over the valid
    prefix), then the row is compacted left (shift-by-one from the drop
    position, zero appended at the end of the valid region).
  - all other rows pass through unchanged.

Device strategy (pure data parallel over 8 NeuronCores):
  - Only odd rows can ever change -> only the 4096 odd rows are processed
    on device (512 rows per core); even rows are copied host-side.
  - Token ids are < 32000, so rows ship to the device as int16 (halves
    HBM traffic).
  - The random matrix r = uniform(key42, (8192, 2048)) is input-independent,
    so its per-row prefix-minimum "record" positions are precomputed on the
    host (max 18 records/row, padded to K=32) and shipped as a tiny constant.
  - Per 128-row tile, work is spread over three engines:
      ACT:    n_tokens = accumulate(sign(x))
      DVE:    thresh = max over k of rec_k*(rec_k < nt), plus the
              (n_tokens < 10 -> no drop) override; in-place predicated
              left-shift-by-one (the compaction)
      DVE:    mask = (iota >= thresh)  (GpSimd is ~15x too slow for this)
"""

import os
import sys

import numpy as np

_TRN_REPO = "/opt/trn_rl_repo"
if os.path.isdir(_TRN_REPO) and _TRN_REPO not in sys.path:  # pragma: no cover
    sys.path.insert(0, _TRN_REPO)

B, S = 8192, 2048
N_CORES = 8
MIN_TOKENS = 10
K = 32  # padded record-position count (measured max is 18)
ROWS_PER_CORE = (B // 2) // N_CORES  # 512 odd rows per core
TILES_PER_CORE = ROWS_PER_CORE // 128  # 4 tiles of [128, 2048]

_cache = {}


def _records_odd() -> np.ndarray:
    """[4096, K] int32: prefix-min record positions of r's odd rows, pad=S."""
    if "rec" in _cache:
        return _cache["rec"]
    import jax

    with jax.default_device(jax.devices("cpu")[0]):
        key = jax.random.key(42)
        r = np.asarray(jax.random.uniform(key, (B, S)))
    r_odd = r[1::2]
    pm = np.minimum.accumulate(r_odd, axis=1)
    is_rec = np.empty(r_odd.shape, dtype=bool)
    is_rec[:, 0] = True
    is_rec[:, 1:] = r_odd[:, 1:] < pm[:, :-1]
    assert int(is_rec.sum(1).max()) <= K
    rec = np.full((B // 2, K), S, np.int32)
    cc = np.cumsum(is_rec, axis=1) - 1
    rows, cols = np.nonzero(is_rec)
    rec[rows, cc[rows, cols]] = cols
    _cache["rec"] = rec
    return rec


def _build_program():
    """Build the single-core Bass program (SPMD across 8 cores)."""
    import concourse.bacc as bacc
    import concourse.mybir as mybir
    import concourse.tile as tile

    i16 = mybir.dt.int16
    i32 = mybir.dt.int32
    f32 = mybir.dt.float32
    Alu = mybir.AluOpType

    # Bacc (not plain Bass): its compile() splits multi-sem waits into the
    # event-semaphore form the TRN2 sequencers require (<=1 wait per inst).
    nc = bacc.Bacc(
        "TRN2",
        target_bir_lowering=False,
        debug=False,
        enable_asserts=False,
        num_devices=N_CORES,
    )
    x_d = nc.dram_tensor("x", [ROWS_PER_CORE, S], i16, kind="ExternalInput").ap()
    # records, interleaved so tile t / partition p reads row t*128+p:
    # host layout [128, TILES_PER_CORE * K]
    rec_d = nc.dram_tensor(
        "rec", [128, TILES_PER_CORE * K], i32, kind="ExternalInput"
    ).ap()
    iota_d = nc.dram_tensor("iota", [128, S], i16, kind="ExternalInput").ap()
    y_d = nc.dram_tensor("y", [ROWS_PER_CORE, S], i16, kind="ExternalOutput").ap()

    with tile.TileContext(nc) as tc:
        with (
            tc.tile_pool(name="const", bufs=1) as const_pool,
            tc.tile_pool(name="xin", bufs=4) as xin_pool,
            tc.tile_pool(name="mask", bufs=4) as mask_pool,
            tc.tile_pool(name="scr", bufs=4) as scr_pool,
            tc.tile_pool(name="small", bufs=8) as small_pool,
        ):
            # x tile 0 first: it gates the whole pipeline
            xts = [
                xin_pool.tile([128, S + 1], i16, tag="xt", name=f"xt{t}")
                for t in range(TILES_PER_CORE)
            ]
            # issue order: x0 (gates the pipeline), then the constants the
            # first tile's DVE chain needs, then the remaining x tiles
            nc.sync.dma_start(xts[0][:, 0:S], x_d[0:128, :])
            iota = const_pool.tile([128, S], i16, tag="iota")
            nc.sync.dma_start(iota[:], iota_d[:])
            rec_sb = const_pool.tile([128, TILES_PER_CORE * K], i32, tag="rec")
            nc.sync.dma_start(rec_sb[:], rec_d[:])
            for t in range(1, TILES_PER_CORE):
                nc.sync.dma_start(
                    xts[t][:, 0:S], x_d[t * 128 : (t + 1) * 128, :]
                )
            for t in range(TILES_PER_CORE):
                nc.vector.memset(xts[t][:, S : S + 1], 0)

            for t in range(TILES_PER_CORE):
                xt = xts[t]

                # n_tokens = sum(sign(x)) on the scalar engine (ids >= 0)
                scr = scr_pool.tile([128, S], i16)
                nt = small_pool.tile([128, 1], f32, tag="nt")
                nc.scalar.activation(
                    scr[:],
                    xt[:, 0:S],
                    mybir.ActivationFunctionType.Sign,
                    accum_out=nt[:],
                )

                # thresh = max( max_k rec_k*(rec_k < nt), (x[:,9]==0)*S )
                rt = rec_sb[:, t * K : (t + 1) * K]
                tmp = small_pool.tile([128, K + 1], i32, tag="tmp")
                nc.vector.scalar_tensor_tensor(
                    tmp[:, 0:K], rt, nt[:], rt, Alu.is_lt, Alu.mult
                )
                nc.vector.tensor_scalar(
                    tmp[:, K : K + 1],
                    xt[:, MIN_TOKENS - 1 : MIN_TOKENS],
                    0.0,
                    float(S),
                    Alu.is_equal,
                    Alu.mult,
                )
                thr = small_pool.tile([128, 1], f32, tag="thr")
                nc.vector.tensor_reduce(
                    thr[:], tmp[:], mybir.AxisListType.X, Alu.max
                )

                # mask = (iota >= thresh) on DVE; predicated in-place shift.
                # (An ACT relu(iota-thr+1) variant for the mask wedged the
                # execution unit on HW — keep the mask on DVE.)
                mk = mask_pool.tile([128, S], i16)
                nc.vector.tensor_scalar(
                    mk[:], iota[:], thr[:], None, Alu.is_ge, Alu.bypass
                )
                nc.vector.copy_predicated(xt[:, 0:S], mk[:], xt[:, 1 : S + 1])
                nc.sync.dma_start(y_d[t * 128 : (t + 1) * 128, :], xt[:, 0:S])

    nc.finalize()
    return nc


def _get_program():
    if "nc" not in _cache:
        _cache["nc"] = _build_program()
    return _cache["nc"]


def _shard_inputs(input_ids: np.ndarray):
    """Per-core in_maps: odd rows of the core's contiguous 1024-row block."""
    rec = _records_odd()
    odd16 = input_ids[1::2].astype(np.int16)  # ids < 32000 fit int16
    iota = np.broadcast_to(np.arange(S, dtype=np.int16), (128, S)).copy()
    in_maps = []
    for c in range(N_CORES):
        sl = slice(c * ROWS_PER_CORE, (c + 1) * ROWS_PER_CORE)
        rec_c = (
            rec[sl]
            .reshape(TILES_PER_CORE, 128, K)
            .transpose(1, 0, 2)
            .reshape(128, TILES_PER_CORE * K)
        )
        in_maps.append(
            {
                "x": np.ascontiguousarray(odd16[sl]),
                "rec": np.ascontiguousarray(rec_c),
                "iota": iota,
            }
        )
    return in_maps


def _run(input_ids: np.ndarray, trace: bool = False):
    from concourse.bass_utils import run_bass_kernel_spmd

    nc = _get_program()
    in_maps = _shard_inputs(input_ids)
    return run_bass_kernel_spmd(nc, in_maps, list(range(N_CORES)), trace=trace)


def kernel(input_ids: np.ndarray) -> np.ndarray:
    input_ids = np.ascontiguousarray(np.asarray(input_ids, dtype=np.int32))
    assert input_ids.shape == (B, S)
    res = _run(input_ids)
    out = input_ids.copy()
    out[1::2] = np.concatenate([m["y"] for m in res.results], axis=0)
    return out


# revision 19
# speedup vs baseline: 4.6645x; 1.0157x over previous
"""Trainium2 Bass kernel for nn_RandomDropout (B=8192, S=2048, int32 ids).

Semantics (training mode of the module):
  - rows with odd batch index and n_tokens >= 10 get ONE random valid
    position dropped (argmin of jax.random.uniform(key 42) over the valid
    prefix), then the row is compacted left (shift-by-one from the drop
    position, zero appended at the end of the valid region).
  - all other rows pass through unchanged.

Device strategy (pure data parallel over 8 NeuronCores):
  - Only odd rows can ever change -> only the 4096 odd rows are processed
    on device (512 rows per core); even rows are copied host-side.
  - Token ids are < 32000, so rows ship to the device as int16 (halves
    HBM traffic).
  - The random matrix r = uniform(key42, (8192, 2048)) is input-independent,
    so its per-row prefix-minimum "record" positions are precomputed on the
    host (max 18 records/row, padded to K=32) and shipped as a tiny constant.
  - Per 128-row tile, work is spread over three engines:
      ACT:    n_tokens = accumulate(sign(x))
      DVE:    thresh = max over k of rec_k*(rec_k < nt), plus the
              (n_tokens < 10 -> no drop) override; in-place predicated
              left-shift-by-one (the compaction)
      DVE:    mask = (iota >= thresh)  (GpSimd is ~15x too slow for this)
"""

import os
import sys

import numpy as np

_TRN_REPO = "/opt/trn_rl_repo"
if os.path.isdir(_TRN_REPO) and _TRN_REPO not in sys.path:  # pragma: no cover
    sys.path.insert(0, _TRN_REPO)

B, S = 8192, 2048
N_CORES = 8
MIN_TOKENS = 10
K = 32  # padded record-position count (measured max is 18)
ROWS_PER_CORE = (B // 2) // N_CORES  # 512 odd rows per core
TILES_PER_CORE = ROWS_PER_CORE // 128  # 4 tiles of [128, 2048]

_cache = {}


def _records_odd() -> np.ndarray:
    """[4096, K] int32: prefix-min record positions of r's odd rows, pad=S."""
    if "rec" in _cache:
        return _cache["rec"]
    import jax

    with jax.default_device(jax.devices("cpu")[0]):
        key = jax.random.key(42)
        r = np.asarray(jax.random.uniform(key, (B, S)))
    r_odd = r[1::2]
    pm = np.minimum.accumulate(r_odd, axis=1)
    is_rec = np.empty(r_odd.shape, dtype=bool)
    is_rec[:, 0] = True
    is_rec[:, 1:] = r_odd[:, 1:] < pm[:, :-1]
    assert int(is_rec.sum(1).max()) <= K
    rec = np.full((B // 2, K), S, np.int32)
    cc = np.cumsum(is_rec, axis=1) - 1
    rows, cols = np.nonzero(is_rec)
    rec[rows, cc[rows, cols]] = cols
    _cache["rec"] = rec
    return rec


def _build_program():
    """Build the single-core Bass program (SPMD across 8 cores)."""
    import concourse.bacc as bacc
    import concourse.mybir as mybir
    import concourse.tile as tile

    i16 = mybir.dt.int16
    i32 = mybir.dt.int32
    f32 = mybir.dt.float32
    Alu = mybir.AluOpType

    # Bacc (not plain Bass): its compile() splits multi-sem waits into the
    # event-semaphore form the TRN2 sequencers require (<=1 wait per inst).
    nc = bacc.Bacc(
        "TRN2",
        target_bir_lowering=False,
        debug=False,
        enable_asserts=False,
        num_devices=N_CORES,
    )
    x_d = nc.dram_tensor("x", [ROWS_PER_CORE, S], i16, kind="ExternalInput").ap()
    # records, interleaved so tile t / partition p reads row t*128+p:
    # host layout [128, TILES_PER_CORE * K]
    rec_d = nc.dram_tensor(
        "rec", [128, TILES_PER_CORE * K], i32, kind="ExternalInput"
    ).ap()
    iota_d = nc.dram_tensor("iota", [128, S], i16, kind="ExternalInput").ap()
    y_d = nc.dram_tensor("y", [ROWS_PER_CORE, S], i16, kind="ExternalOutput").ap()

    with tile.TileContext(nc) as tc:
        with (
            tc.tile_pool(name="const", bufs=1) as const_pool,
            tc.tile_pool(name="xin", bufs=4) as xin_pool,
            tc.tile_pool(name="mask", bufs=4) as mask_pool,
            tc.tile_pool(name="scr", bufs=4) as scr_pool,
            tc.tile_pool(name="small", bufs=8) as small_pool,
        ):
            # x tile 0 first: it gates the whole pipeline
            xts = [
                xin_pool.tile([128, S + 1], i16, tag="xt", name=f"xt{t}")
                for t in range(TILES_PER_CORE)
            ]
            # issue order: x0 (gates the pipeline), then the constants the
            # first tile's DVE chain needs, then the remaining x tiles
            nc.sync.dma_start(xts[0][:, 0:S], x_d[0:128, :])
            iota = const_pool.tile([128, S], i16, tag="iota")
            nc.sync.dma_start(iota[:], iota_d[:])
            rec_sb = const_pool.tile([128, TILES_PER_CORE * K], i32, tag="rec")
            nc.sync.dma_start(rec_sb[:], rec_d[:])
            for t in range(1, TILES_PER_CORE):
                nc.sync.dma_start(
                    xts[t][:, 0:S], x_d[t * 128 : (t + 1) * 128, :]
                )
            for t in range(TILES_PER_CORE):
                nc.vector.memset(xts[t][:, S : S + 1], 0)

            for t in range(TILES_PER_CORE):
                xt = xts[t]

                # n_tokens = count(x > 0). Tile 0 computes it on DVE (starts
                # as soon as x0 lands, ~4.5us before ACT's sign0 would hand
                # over); tiles 1-3 use the scalar engine, which stays ahead.
                scr = scr_pool.tile([128, S], i16)
                nt = small_pool.tile([128, 1], f32, tag="nt")
                if t == 0:
                    nc.vector.tensor_scalar(
                        scr[:], xt[:, 0:S], 0.0, None, Alu.is_gt, Alu.add,
                        accum_out=nt[:],
                    )
                else:
                    nc.scalar.activation(
                        scr[:],
                        xt[:, 0:S],
                        mybir.ActivationFunctionType.Sign,
                        accum_out=nt[:],
                    )

                # thresh = max( max_k rec_k*(rec_k < nt), (x[:,9]==0)*S )
                rt = rec_sb[:, t * K : (t + 1) * K]
                tmp = small_pool.tile([128, K + 1], i32, tag="tmp")
                nc.vector.scalar_tensor_tensor(
                    tmp[:, 0:K], rt, nt[:], rt, Alu.is_lt, Alu.mult
                )
                nc.vector.tensor_scalar(
                    tmp[:, K : K + 1],
                    xt[:, MIN_TOKENS - 1 : MIN_TOKENS],
                    0.0,
                    float(S),
                    Alu.is_equal,
                    Alu.mult,
                )
                thr = small_pool.tile([128, 1], f32, tag="thr")
                nc.vector.tensor_reduce(
                    thr[:], tmp[:], mybir.AxisListType.X, Alu.max
                )

                # mask = (iota >= thresh) on DVE; predicated in-place shift.
                # (An ACT relu(iota-thr+1) variant for the mask wedged the
                # execution unit on HW — keep the mask on DVE.)
                mk = mask_pool.tile([128, S], i16)
                nc.vector.tensor_scalar(
                    mk[:], iota[:], thr[:], None, Alu.is_ge, Alu.bypass
                )
                nc.vector.copy_predicated(xt[:, 0:S], mk[:], xt[:, 1 : S + 1])
                nc.sync.dma_start(y_d[t * 128 : (t + 1) * 128, :], xt[:, 0:S])

    nc.finalize()
    return nc


def _get_program():
    if "nc" not in _cache:
        _cache["nc"] = _build_program()
    return _cache["nc"]


def _shard_inputs(input_ids: np.ndarray):
    """Per-core in_maps: odd rows of the core's contiguous 1024-row block."""
    rec = _records_odd()
    odd16 = input_ids[1::2].astype(np.int16)  # ids < 32000 fit int16
    iota = np.broadcast_to(np.arange(S, dtype=np.int16), (128, S)).copy()
    in_maps = []
    for c in range(N_CORES):
        sl = slice(c * ROWS_PER_CORE, (c + 1) * ROWS_PER_CORE)
        rec_c = (
            rec[sl]
            .reshape(TILES_PER_CORE, 128, K)
            .transpose(1, 0, 2)
            .reshape(128, TILES_PER_CORE * K)
        )
        in_maps.append(
            {
                "x": np.ascontiguousarray(odd16[sl]),
                "rec": np.ascontiguousarray(rec_c),
                "iota": iota,
            }
        )
    return in_maps


def _run(input_ids: np.ndarray, trace: bool = False):
    from concourse.bass_utils import run_bass_kernel_spmd

    nc = _get_program()
    in_maps = _shard_inputs(input_ids)
    return run_bass_kernel_spmd(nc, in_maps, list(range(N_CORES)), trace=trace)


def kernel(input_ids: np.ndarray) -> np.ndarray:
    input_ids = np.ascontiguousarray(np.asarray(input_ids, dtype=np.int32))
    assert input_ids.shape == (B, S)
    res = _run(input_ids)
    out = input_ids.copy()
    out[1::2] = np.concatenate([m["y"] for m in res.results], axis=0)
    return out
